# revision 33
# baseline (speedup 1.0000x reference)
"""BiMamba Trainium2 kernel.

Sharding: 8 cores = (batch 2) x (direction 2) x (head-half 2). Each core runs an
identical SPMD Bass program on its slice: x[b]^T (time-flipped for bwd), in_proj
rows for its 12 heads (+ shared B/C rows). Per-core output: unnormalized
projected partial (2048, 768) + partial sum-of-squares. The RMSNorm rsqrt
commutes with the linear projection, so the host applies it to the summed
partials, then adds proj bias.

Scan: chunked SSD, chunk=128:
  y_t = sum_{s<=t} (B_s.C_t) exp(Acum_t - Acum_s) dt_s x_s + exp(Acum_t) (C_t.h_prev)
Decay matrix: exact-fp32 cumsum via triangular matmul; the (s,t) plane
D[s,t] = (logdt_s - Acum_s) + Acum_t is built by one K=39 bf16 matmul per chunk
(3 ones rows x 3-way-split Acum_t + 36 blocked-ones rows x 3-way-split per-head
bias); ACT exp; mask+CB on DVE.
"""
import numpy as np
from contextlib import ExitStack

import concourse.bass as bass
import concourse.tile as tile
from concourse import bacc, mybir
from concourse.bass_utils import run_bass_kernel_spmd
from concourse.masks import make_identity

FP32 = mybir.dt.float32
FP32R = mybir.dt.float32r
BF16 = mybir.dt.bfloat16
AF = mybir.ActivationFunctionType
ALU = mybir.AluOpType

D_MODEL = 768
D_STATE = 16
HEADDIM = 64
D_CONV = 4
SEQ = 2048
NH = 12                  # heads per core
HH = NH * HEADDIM        # 768 x-channels per core
CMJ = HH + NH + 2 * D_STATE   # 812 c-major feats: [x 768 | dt 12 | B 16 | C 16]
TMJ = HH + NH            # 780 t-major feats: [z 768 | dt 12]
CH = 128
NCHUNK = SEQ // CH       # 16
TB = 256                 # time block
NTB = SEQ // TB
CPB = TB // CH           # 2
NKT = 6                  # d_model k-tiles
EPS = 1e-5
P = 128


def _rep(ap_tile, inner, outer_count, inner_count, outer_step, inner_step):
    """free-pattern AP helper on a 2D tile: [[pstep,P],[outer],[inner]]"""
    return bass.AP(tensor=ap_tile.tensor, offset=ap_tile.offset,
                   ap=[[ap_tile.ap[0][0], ap_tile.ap[0][1]],
                       [outer_step, outer_count], [inner_step, inner_count]])


def _pbcast(src, parts):
    """broadcast a (1, N) AP across `parts` partitions (DMA source only)."""
    assert src.ap[0][1] == 1
    return bass.AP(tensor=src.tensor, offset=src.offset,
                   ap=[src.ap[0], [0, parts]] + [list(d) for d in src.ap[1:]])


def build_program():
    nc = bacc.Bacc("TRN2", target_bir_lowering=False, debug=False, num_devices=8)

    def din(name, shape, dt=FP32):
        return nc.dram_tensor(name, shape, dt, kind="ExternalInput").ap()

    d_xT = din("xT", (D_MODEL, SEQ), FP32R)
    d_Wc = din("Wc", (D_MODEL, CMJ), FP32R)
    d_Wt = din("Wt", (D_MODEL, TMJ), FP32R)
    d_DIAGW = din("DIAGW", (D_CONV, NKT, P, P), FP32R)     # x-part diag tiles
    d_DIAGB = din("DIAGB", (D_CONV, P, D_STATE), FP32R)    # B: in-rows 780..795 -> out 0..15
    d_DIAGC = din("DIAGC", (D_CONV, P, D_STATE), FP32R)    # C: in-rows 796..811 -> out 0..15
    d_CONVBX = din("CONVBX", (P, NKT))                      # x-part conv bias per c-tile
    d_CONVBB = din("CONVBB", (D_STATE, 1))
    d_CONVBC = din("CONVBC", (D_STATE, 1))
    d_DTBIAS = din("DTBIAS", (NH, 1))
    d_DTB_BC = din("DTB_BC", (P, NH))
    d_ANEG_BC = din("ANEG_BC", (P, NH))
    d_TRI = din("TRI", (P, P))                              # tri[s,t]=1 if s<=t
    d_ONES3 = din("ONES3", (3, P), BF16)
    d_RHSC = din("RHSC", (3 * NH, NH * CH), BF16)
    d_DPBIG = din("DPBIG", (P, HH))
    d_WCOMB = din("WCOMB", (HH, D_MODEL), FP32R)
    d_OUT1 = nc.dram_tensor("OUT1", (SEQ, D_MODEL), FP32, kind="ExternalOutput").ap()
    d_OUT2 = nc.dram_tensor("OUT2", (P, NCHUNK), FP32, kind="ExternalOutput").ap()

    with tile.TileContext(nc, trace_sim=False) as tc, ExitStack() as ctx:
        const = ctx.enter_context(tc.tile_pool(name="const", bufs=1))
        wgt = ctx.enter_context(tc.tile_pool(name="wgt", bufs=1))
        seqp = ctx.enter_context(tc.tile_pool(name="seqp", bufs=1))
        blk1 = ctx.enter_context(tc.tile_pool(name="blk1", bufs=1))
        blk2 = ctx.enter_context(tc.tile_pool(name="blk2", bufs=2))
        chk = ctx.enter_context(tc.tile_pool(name="chk", bufs=1))
        st = ctx.enter_context(tc.tile_pool(name="st", bufs=2))
        ps = ctx.enter_context(tc.tile_pool(name="ps", bufs=1, space="PSUM"))
        psA = ctx.enter_context(tc.tile_pool(name="psA", bufs=2, space="PSUM"))

        # ---- constants ----
        tri = const.tile([P, P], FP32); nc.sync.dma_start(tri[:], d_TRI)
        dpbig = const.tile([P, HH], FP32); nc.sync.dma_start(dpbig[:], d_DPBIG)
        convbx = const.tile([P, NKT], FP32); nc.sync.dma_start(convbx[:], d_CONVBX)
        convbb = const.tile([D_STATE, 1], FP32); nc.sync.dma_start(convbb[:], d_CONVBB)
        convbc = const.tile([D_STATE, 1], FP32); nc.sync.dma_start(convbc[:], d_CONVBC)
        dtbias = const.tile([NH, 1], FP32); nc.sync.dma_start(dtbias[:], d_DTBIAS)
        dtb_bc = const.tile([P, NH], FP32); nc.sync.dma_start(dtb_bc[:], d_DTB_BC)
        aneg_bc = const.tile([P, NH], FP32); nc.sync.dma_start(aneg_bc[:], d_ANEG_BC)
        idn = const.tile([P, P], FP32); make_identity(nc, idn)
        b25 = const.tile([P, 1], FP32); nc.vector.memset(b25[:], 25.0)
        idnr = const.tile([P, P], FP32R); nc.vector.tensor_copy(idnr[:], idn[:])

        wc = []
        for kt in range(NKT):
            w = wgt.tile([P, CMJ], FP32R, tag=f"wc{kt}")
            nc.sync.dma_start(w[:], d_Wc[kt * P:(kt + 1) * P, :])
            wc.append(w)
        wt = []
        for kt in range(NKT):
            w = wgt.tile([P, TMJ], FP32R, tag=f"wt{kt}")
            nc.sync.dma_start(w[:], d_Wt[kt * P:(kt + 1) * P, :])
            wt.append(w)
        diagw = [[None] * NKT for _ in range(D_CONV)]
        for k in range(D_CONV):
            for ct in range(NKT):
                w = wgt.tile([P, P], FP32R, tag=f"dw{k}_{ct}")
                nc.sync.dma_start(w[:], d_DIAGW[k, ct])
                diagw[k][ct] = w
        diagb = []; diagc = []
        for k in range(D_CONV):
            w = wgt.tile([P, D_STATE], FP32R, tag=f"db{k}")
            nc.sync.dma_start(w[:], d_DIAGB[k]); diagb.append(w)
            w = wgt.tile([P, D_STATE], FP32R, tag=f"dc{k}")
            nc.sync.dma_start(w[:], d_DIAGC[k]); diagc.append(w)
        wcomb = []
        for ct in range(NKT):
            w = wgt.tile([P, D_MODEL], FP32R, tag=f"wo{ct}")
            nc.sync.dma_start(w[:], d_WCOMB[ct * P:(ct + 1) * P, :])
            wcomb.append(w)

        ssqall = seqp.tile([P, NCHUNK], FP32)
        lhsD = seqp.tile([3 + 3 * NH, P], BF16, tag="lhsD")
        nc.gpsimd.dma_start(lhsD[0:3, :], d_ONES3)
        rhsD = seqp.tile([3 + 3 * NH, NH * CH], BF16, tag="rhsD")
        nc.gpsimd.dma_start(rhsD[3:, :], d_RHSC)
        hN = None
        xbc = None

        for tb in range(NTB):
            t0 = tb * TB
            xtb = []
            for kt in range(NKT):
                x = blk1.tile([P, TB], FP32R, tag=f"xtb{kt}")
                nc.sync.dma_start(x[:], d_xT[kt * P:(kt + 1) * P, t0:t0 + TB])
                xtb.append(x)

            # ---- in_proj c-major (conv input tiles, left-pad 3) ----
            xbc_prev = xbc if tb > 0 else None
            xbc = []
            dtC = blk1.tile([NH, TB], FP32, tag="dtC")
            for ct in range(NKT + 1):
                cw = P if ct < NKT else CMJ - NKT * P   # 44 in last tile
                p = psA.tile([P, 512], FP32, tag="psA")
                for kt in range(NKT):
                    nc.tensor.matmul(p[:cw, 0:TB], wc[kt][:, ct * P:ct * P + cw],
                                     xtb[kt][:], start=(kt == 0), stop=(kt == NKT - 1))
                xb = blk2.tile([P, TB + 3], FP32R, tag=f"xbc{ct}")
                if xbc_prev is None:
                    nc.vector.memset(xb[:cw, 0:3].bitcast(FP32), 0.0)
                else:
                    nc.vector.tensor_copy(xb[:cw, 0:3], xbc_prev[ct][:cw, TB:TB + 3])
                nc.scalar.copy(xb[:cw, 3:], p[:cw, 0:TB])
                if ct == NKT:  # dt rows 0..11 of this tile, exact fp32 from psum
                    nc.vector.tensor_copy(dtC[:], p[0:NH, 0:TB])
                xbc.append(xb)

            # ---- in_proj t-major: z | dt ----
            ztiles = []
            dtall = blk1.tile([P, CPB * NH], FP32, tag="dtall")
            for tt in range(CPB):
                z = blk1.tile([P, HH], FP32, tag=f"zt{tt}")
                for nb in range(2):
                    f0 = nb * 390
                    p = psA.tile([P, 512], FP32, tag="psA")
                    for kt in range(NKT):
                        nc.tensor.matmul(p[:, 0:390], xtb[kt][:, tt * P:(tt + 1) * P],
                                         wt[kt][:, f0:f0 + 390],
                                         start=(kt == 0), stop=(kt == NKT - 1))
                    if nb == 0:
                        nc.scalar.copy(z[:, 0:390], p[:, 0:390])
                    else:
                        nc.scalar.copy(z[:, 390:HH], p[:, 0:HH - 390])
                        nc.vector.tensor_copy(dtall[:, tt * NH:(tt + 1) * NH],
                                              p[:, HH - 390:HH - 390 + NH])
                ztiles.append(z)

            # ---- conv (diag matmuls) + silu ----
            xsil = []
            for ct in range(NKT):
                p = psA.tile([P, 512], FP32, tag="psA")
                for k in range(D_CONV):
                    nc.tensor.matmul(p[:, 0:TB], diagw[k][ct][:], xbc[ct][:, k:k + TB],
                                     start=(k == 0), stop=(k == D_CONV - 1))
                xsl = blk1.tile([P, TB], FP32R, tag=f"xsil{ct}")
                nc.scalar.activation(xsl[:], p[:, 0:TB], AF.Silu,
                                     bias=convbx[:, ct:ct + 1], scale=1.0)
                xsil.append(xsl)
            bsil = blk1.tile([D_STATE, TB], FP32R, tag="bsil")
            csil = blk1.tile([D_STATE, TB], FP32R, tag="csil")
            for dst, dg, bias in ((bsil, diagb, convbb), (csil, diagc, convbc)):
                p = psA.tile([P, 512], FP32, tag="psA")
                for k in range(D_CONV):
                    nc.tensor.matmul(p[:D_STATE, 0:TB], dg[k][0:44, :], xbc[NKT][0:44, k:k + TB],
                                     start=(k == 0), stop=(k == D_CONV - 1))
                nc.scalar.activation(dst[:], p[:D_STATE, 0:TB], AF.Silu,
                                     bias=bias[:], scale=1.0)

            # ---- transpose x + B to s-major ----
            xs_tiles = []
            for tt in range(CPB):
                xst = blk2.tile([P, HH + D_STATE], FP32R, tag=f"xst{tt}")
                for g in range(2):  # two groups of 3 transposes + (B on 2nd)
                    pt = psA.tile([P, 512], FP32, tag="psA")
                    for k in range(3):
                        ct = g * 3 + k
                        nc.tensor.transpose(pt[:, k * P:(k + 1) * P].bitcast(FP32R),
                                            xsil[ct][:, tt * P:(tt + 1) * P], idnr[:])
                    if g == 1:
                        nc.tensor.transpose(pt[:, 3 * P:3 * P + D_STATE].bitcast(FP32R),
                                            bsil[:, tt * P:(tt + 1) * P],
                                            idnr[0:D_STATE, 0:D_STATE])
                        nc.scalar.copy(xst[:, g * 384:g * 384 + 384 + D_STATE],
                                       pt[:, 0:384 + D_STATE])
                    else:
                        nc.scalar.copy(xst[:, 0:384], pt[:, 0:384])
                xs_tiles.append(xst)

            # ---- z silu (batched; keeps ACT on one table inside chunk loop) ----
            sztiles = []
            for tt in range(CPB):
                sz = chk.tile([P, HH], FP32, tag=f"sz{tt}")
                nc.scalar.activation(sz[:], ztiles[tt][:], AF.Silu)
                sztiles.append(sz)

            # ---- dt path (block-batched) ----
            W = CPB * NH
            tmp = chk.tile([P, W], FP32, tag="dtt")
            nc.vector.tensor_tensor(tmp[:], dtall[:],
                                    _rep(dtb_bc, None, CPB, NH, 0, 1), ALU.add)
            spe = chk.tile([P, W], FP32, tag="spe")
            nc.scalar.activation(spe[:], tmp[:], AF.Exp)
            sp = chk.tile([P, W], FP32, tag="sp")
            nc.scalar.activation(sp[:], spe[:], AF.Ln, bias=1.0)
            logda = chk.tile([P, W], FP32, tag="logda")
            nc.vector.tensor_tensor(logda[:], sp[:],
                                    _rep(aneg_bc, None, CPB, NH, 0, 1), ALU.mult)
            logdt = chk.tile([P, W], FP32, tag="logdt")
            nc.scalar.activation(logdt[:], sp[:], AF.Ln)
            acum = chk.tile([P, W], FP32, tag="acum")
            acumC = chk.tile([NH, TB], FP32, tag="acumC")
            for i in range(CPB):
                pa = psA.tile([P, 512], FP32, tag="psA")
                nc.tensor.matmul(pa[:, 0:NH], tri[:], logda[:, i * NH:(i + 1) * NH],
                                 start=True, stop=True)
                nc.vector.tensor_copy(acum[:, i * NH:(i + 1) * NH], pa[:, 0:NH])
                pc = psA.tile([P, 512], FP32, tag="psA")
                nc.tensor.matmul(pc[:NH, 0:P], logda[:, i * NH:(i + 1) * NH], tri[:],
                                 start=True, stop=True)
                nc.vector.tensor_copy(acumC[:, i * P:(i + 1) * P], pc[:NH, 0:P])
            expac = chk.tile([P, W], FP32, tag="expac")
            nc.scalar.activation(expac[:], acum[:], AF.Exp)
            spce = chk.tile([NH, TB], FP32, tag="spce")
            nc.scalar.activation(spce[:], dtC[:], AF.Exp, bias=dtbias[:], scale=1.0)
            spc = chk.tile([NH, TB], FP32, tag="spc")
            nc.scalar.activation(spc[:], spce[:], AF.Ln, bias=1.0)
            ldc = chk.tile([NH, TB], FP32, tag="ldc")
            nc.scalar.activation(ldc[:], spc[:], AF.Ln)
            nc.vector.tensor_sub(ldc[:], ldc[:], acumC[:])
            splits = {}
            for nm, src in (("ac", acumC), ("ld", ldc)):
                r1 = chk.tile([NH, TB], FP32, tag="r1")
                r2 = chk.tile([NH, TB], FP32, tag="r2")
                h_ = chk.tile([NH, TB], BF16, tag=nm + "H")
                m_ = chk.tile([NH, TB], BF16, tag=nm + "M")
                l_ = chk.tile([NH, TB], BF16, tag=nm + "L")
                nc.vector.tensor_copy(h_[:], src[:])
                nc.vector.tensor_sub(r1[:], src[:], h_[:])
                nc.vector.tensor_copy(m_[:], r1[:])
                nc.vector.tensor_sub(r2[:], r1[:], m_[:])
                nc.vector.tensor_copy(l_[:], r2[:])
                splits[nm] = (h_, m_, l_)

            # ---- chunks ----
            for i in range(CPB):
                ci = (t0 // P) + i
                xst = xs_tiles[i]
                acs = slice(i * NH, (i + 1) * NH)
                ccs = slice(i * P, (i + 1) * P)

                # wS = exp(logdt - Acum + Aend)
                ldcs = chk.tile([P, NH], FP32, tag="ldcs")
                nc.vector.tensor_sub(ldcs[:], logdt[:, acs], acum[:, acs])
                aendbc = chk.tile([P, NH], FP32, tag="aendbc")
                nc.gpsimd.dma_start(aendbc[:], _pbcast(acum[127:128, acs], P))
                ws = chk.tile([P, NH], FP32, tag="ws")
                nc.vector.tensor_tensor(ws[:], ldcs[:], aendbc[:], ALU.add)
                nc.scalar.activation(ws[:], ws[:], AF.Exp)
                bd = chk.tile([P, NH * D_STATE], FP32R, tag="bd")
                nc.vector.tensor_tensor(
                    bd[:],
                    bass.AP(tensor=xst.tensor, offset=xst.offset + HH,
                            ap=[[xst.ap[0][0], P], [0, NH], [1, D_STATE]]),
                    bass.AP(tensor=ws.tensor, offset=ws.offset,
                            ap=[[ws.ap[0][0], P], [1, NH], [0, D_STATE]]),
                    ALU.mult)

                # D staging (rows 3.. persistent consts; rows 0-2 per chunk)
                for j in range(3):
                    nc.gpsimd.dma_start(lhsD[3 + j * NH:3 + (j + 1) * NH, :],
                                      splits["ld"][j][:, ccs])
                for j in range(3):
                    dst = bass.AP(tensor=rhsD.tensor, offset=rhsD.offset + j * rhsD.ap[0][0],
                                  ap=[[rhsD.ap[0][0], 1], [CH, NH], [1, CH]])
                    nc.gpsimd.dma_start(dst, splits["ac"][j][:, ccs])

                pcbt = psA.tile([P, 512], FP32, tag="psA")
                nc.tensor.matmul(pcbt[:, 0:P], bsil[:, ccs], csil[:, ccs], start=True, stop=True)
                cbtm = chk.tile([P, P], FP32, tag="cbtm")
                nc.vector.tensor_tensor(cbtm[:], pcbt[:, 0:P], tri[:], ALU.mult)

                mall = chk.tile([P, NH * CH], FP32R, tag="mall")
                lall = chk.tile([P, NH * CH], FP32, tag="lall")
                for nb in range(3):
                    pd = psA.tile([P, 512], FP32, tag="psA")
                    nc.tensor.matmul(pd[:], lhsD[:], rhsD[:, nb * 512:(nb + 1) * 512],
                                     start=True, stop=True)
                    sl = lall[:, nb * 512:(nb + 1) * 512]
                    nc.vector.tensor_scalar_min(sl, pd[:], 25.0)
                    nc.scalar.activation(sl, sl, AF.Exp)
                nc.vector.tensor_tensor(mall[:], _rep(cbtm, None, NH, CH, 0, 1),
                                        lall[:], ALU.mult)

                hN_prev = hN
                py2 = None
                if hN_prev is not None:
                    py2 = ps.tile([P, HH], FP32, tag="psY2")
                    nc.tensor.matmul(py2[:, 0:512], csil[:, ccs], hN_prev[:, 0:512],
                                     start=True, stop=True)
                    nc.tensor.matmul(py2[:, 512:HH], csil[:, ccs], hN_prev[:, 512:HH],
                                     start=True, stop=True)

                py = ps.tile([P, HH], FP32, tag="psY")
                for h in range(NH):
                    nc.tensor.matmul(py[:, h * 64:(h + 1) * 64],
                                     mall[:, h * CH:(h + 1) * CH],
                                     xst[:, h * 64:(h + 1) * 64], start=True, stop=True)
                pst = ps.tile([D_STATE, NH * HEADDIM], FP32, tag="psSt")
                for h in range(NH):
                    nc.tensor.matmul(pst[:, h * 64:(h + 1) * 64],
                                     bd[:, h * D_STATE:(h + 1) * D_STATE],
                                     xst[:, h * 64:(h + 1) * 64], start=True, stop=True)

                hN_new = st.tile([D_STATE, NH * HEADDIM], FP32R, tag="hN")
                if hN_prev is None:
                    nc.vector.tensor_copy(hN_new[:], pst[:])
                else:
                    eae = chk.tile([1, NH], FP32, tag="eae")
                    nc.scalar.activation(eae[:], aendbc[0:1, :], AF.Exp)
                    eaebc = chk.tile([D_STATE, NH], FP32, tag="eaebc")
                    nc.gpsimd.dma_start(eaebc[:], _pbcast(eae[:], D_STATE))
                    tmp1 = chk.tile([D_STATE, HH], FP32, tag="tmp1")
                    nc.vector.tensor_tensor(
                        tmp1[:], hN_prev[:],
                        bass.AP(tensor=eaebc.tensor, offset=eaebc.offset,
                                ap=[[eaebc.ap[0][0], D_STATE], [1, NH], [0, HEADDIM]]),
                        ALU.mult)
                    nc.vector.tensor_tensor(hN_new[:], tmp1[:], pst[:], ALU.add)
                hN = hN_new

                # epilogue
                e1 = chk.tile([P, HH], FP32, tag="e1")
                if py2 is not None:
                    nc.vector.tensor_tensor(
                        e1[:], py2[:],
                        bass.AP(tensor=expac.tensor, offset=expac.offset + i * NH,
                                ap=[[expac.ap[0][0], P], [1, NH], [0, HEADDIM]]),
                        ALU.mult)
                    nc.vector.tensor_tensor(e1[:], e1[:], py[:], ALU.add)
                else:
                    nc.vector.tensor_copy(e1[:], py[:])
                e4 = chk.tile([P, HH], FP32, tag="e4")
                nc.gpsimd.tensor_tensor(e4[:], xst[:, 0:HH], dpbig[:], ALU.mult)
                nc.vector.tensor_tensor(e4[:], e4[:], e1[:], ALU.add)
                yg = chk.tile([P, HH], FP32, tag="yg")
                nc.vector.tensor_tensor(yg[:], e4[:], sztiles[i][:], ALU.mult)
                sqs = chk.tile([P, HH], FP32, tag="sqs")
                nc.scalar.activation(sqs[:], yg[:], AF.Square,
                                     accum_out=ssqall[:, ci:ci + 1])

                pw = ps.tile([P, D_MODEL], FP32, tag="psY")
                ygts = []
                for g in range(2):
                    ptr = psA.tile([P, 512], FP32, tag="psA")
                    for k in range(3):
                        ct = g * 3 + k
                        nc.tensor.transpose(ptr[:, k * P:(k + 1) * P], yg[:, ct * P:(ct + 1) * P], idn[:])
                    ygt = chk.tile([P, 384], FP32R, tag=f"ygt{g}")
                    nc.scalar.copy(ygt[:], ptr[:, 0:384])
                    ygts.append(ygt)
                for ct in range(NKT):
                    ygt_sl = ygts[ct // 3][:, (ct % 3) * P:(ct % 3 + 1) * P]
                    nc.tensor.matmul(pw[:, 0:512], ygt_sl, wcomb[ct][:, 0:512],
                                     start=(ct == 0), stop=(ct == NKT - 1))
                    nc.tensor.matmul(pw[:, 512:D_MODEL], ygt_sl, wcomb[ct][:, 512:D_MODEL],
                                     start=(ct == 0), stop=(ct == NKT - 1))
                o1 = chk.tile([P, D_MODEL], FP32, tag="o1")
                nc.scalar.copy(o1[:], pw[:])
                nc.sync.dma_start(d_OUT1[ci * P:(ci + 1) * P, :], o1[:])

        nc.sync.dma_start(d_OUT2, ssqall[:])

    nc.compile()
    return nc


# ================= host side =================

def _prep_core_inputs(x_b_T, in_w, conv_w, conv_b, dt_bias, A_log, Dp, norm_w,
                      out_w, proj_w_dir, hh):
    import ml_dtypes
    D_INNER = 1536
    zsel = slice(hh * HH, (hh + 1) * HH)
    xsel = slice(D_INNER + hh * HH, D_INNER + (hh + 1) * HH)
    Bsel = slice(2 * D_INNER, 2 * D_INNER + 16)
    Csel = slice(2 * D_INNER + 16, 2 * D_INNER + 32)
    dtsel = slice(2 * D_INNER + 32 + hh * NH, 2 * D_INNER + 32 + (hh + 1) * NH)

    # c-major rows: [x 768 | dt 12 | B 16 | C 16]
    Wc_rows = np.concatenate([in_w[xsel], in_w[dtsel], in_w[Bsel], in_w[Csel]], 0)
    Wt_rows = np.concatenate([in_w[zsel], in_w[dtsel]], 0)

    cwx = conv_w[hh * HH:(hh + 1) * HH]          # (768, 4) x-part
    cbx = conv_b[hh * HH:(hh + 1) * HH]
    cwB = conv_w[D_INNER:D_INNER + 16]
    cbB = conv_b[D_INNER:D_INNER + 16]
    cwC = conv_w[D_INNER + 16:D_INNER + 32]
    cbC = conv_b[D_INNER + 16:D_INNER + 32]

    DIAGW = np.zeros((D_CONV, NKT, P, P), np.float32)
    for k in range(D_CONV):
        for ct in range(NKT):
            DIAGW[k, ct][np.arange(P), np.arange(P)] = cwx[ct * P:(ct + 1) * P, k]
    DIAGB = np.zeros((D_CONV, P, D_STATE), np.float32)
    DIAGC = np.zeros((D_CONV, P, D_STATE), np.float32)
    for k in range(D_CONV):
        DIAGB[k][NH + np.arange(16), np.arange(16)] = cwB[:, k]       # in-rows 12..27
        DIAGC[k][NH + 16 + np.arange(16), np.arange(16)] = cwC[:, k]  # in-rows 28..43
    CONVBX = np.zeros((P, NKT), np.float32)
    for ct in range(NKT):
        CONVBX[:, ct] = cbx[ct * P:(ct + 1) * P]

    a_neg = -np.exp(A_log[hh * NH:(hh + 1) * NH]).astype(np.float32)
    dtb = dt_bias[hh * NH:(hh + 1) * NH].astype(np.float32)
    TRIm = np.triu(np.ones((P, P), np.float32))
    RHSC = np.zeros((3 * NH, NH * CH), np.float32)
    for j in range(3):
        for h in range(NH):
            RHSC[j * NH + h, h * CH:(h + 1) * CH] = 1.0
    DPBIG = np.repeat(Dp[hh * NH:(hh + 1) * NH].astype(np.float32), HEADDIM)[None, :] \
        .repeat(P, 0).copy()
    ow = (out_w * norm_w[None, :]).astype(np.float32)
    WCOMB = np.ascontiguousarray((proj_w_dir @ ow)[:, hh * HH:(hh + 1) * HH].T)

    bf = lambda a: np.ascontiguousarray(a).astype(ml_dtypes.bfloat16)
    f = np.ascontiguousarray
    return {
        "xT": f(x_b_T.astype(np.float32)),
        "Wc": f(Wc_rows.T.astype(np.float32)),
        "Wt": f(Wt_rows.T.astype(np.float32)),
        "DIAGW": DIAGW, "DIAGB": DIAGB, "DIAGC": DIAGC,
        "CONVBX": CONVBX,
        "CONVBB": f(cbB.astype(np.float32)[:, None]),
        "CONVBC": f(cbC.astype(np.float32)[:, None]),
        "DTBIAS": f(dtb[:, None]),
        "DTB_BC": f(np.repeat(dtb[None, :], P, 0)),
        "ANEG_BC": f(np.repeat(a_neg[None, :], P, 0)),
        "TRI": TRIm,
        "ONES3": bf(np.ones((3, P), np.float32)),
        "RHSC": bf(RHSC),
        "DPBIG": DPBIG,
        "WCOMB": f(WCOMB.astype(np.float32)),
    }


def make_in_maps(inputs):
    x = np.asarray(inputs["x"], np.float32)
    proj_w = np.asarray(inputs["proj_w"], np.float32)
    in_maps, core_meta = [], []
    for b in range(2):
        for d, pref in ((0, "f_"), (1, "b_")):
            xb = x[b] if d == 0 else x[b][::-1]
            for hh in range(2):
                g = lambda n: np.asarray(inputs[pref + n], np.float32)
                im = _prep_core_inputs(
                    np.ascontiguousarray(xb.T), g("in_w"), g("conv_w"), g("conv_b"),
                    g("dt_bias"), g("A_log"), g("Dp"), g("norm_w"), g("out_w"),
                    proj_w[:, d * D_MODEL:(d + 1) * D_MODEL], hh)
                in_maps.append(im)
                core_meta.append((b, d, hh))
    return in_maps, core_meta


def combine_outputs(results, core_meta, proj_b):
    out = np.zeros((2, SEQ, D_MODEL), np.float32)
    for b in range(2):
        for d in range(2):
            idx = [i for i, (bb, dd, _) in enumerate(core_meta) if bb == b and dd == d]
            part = sum(results[i]["OUT1"] for i in idx)
            ssq = sum(results[i]["OUT2"] for i in idx)       # (128, 16)
            ssq_t = ssq.T.reshape(SEQ)                        # t = ci*128 + p
            s = 1.0 / np.sqrt(ssq_t / 1536.0 + EPS)
            contrib = part * s[:, None]
            if d == 1:
                contrib = contrib[::-1]
            out[b] += contrib
    out += np.asarray(proj_b, np.float32)[None, None, :]
    return out


_NC_CACHE = {}


def kernel(**inputs):
    in_maps, core_meta = make_in_maps(inputs)
    if "nc" not in _NC_CACHE:
        _NC_CACHE["nc"] = build_program()
    nc = _NC_CACHE["nc"]
    res = run_bass_kernel_spmd(nc, in_maps, list(range(8)))
    return combine_outputs(res.results, core_meta, inputs["proj_b"])
